# revision 1
# baseline (speedup 1.0000x reference)
"""Trainium2 Bass kernel for nn_AttentionTIE (TIE-style edge-LayerNorm attention).

Sharding: 8 cores = (batch b = core//2) x (receiver-row half = core%2).
Each core computes the full v_sender for its batch, attention for its 1536
receiver rows, and the three projected outputs for those rows.

Algorithm per core (all shapes [partition, free]):
  v_sT  = W_send @ xT + W_mem @ sendT + res_sT            [C, N]
  v_rT  = W_recv @ xT_own + W_mem @ recvT_own + res_rT    [C, No]
  qT    = (W_q*scale) @ xT_own                            [C, No]
  std2[i,j] = u_i + w_j + (2/D) v_r.v_s - 2 m_r m_s + eps  (aug rank-2 matmul,
              u_i via ACT sqrt bias)
  score[i,j] = q.v_s + (alpha_i) - sumq_i m_s_j - M*mask   (aug rank-2 matmul +
              identity x maskbias matmul)
  T = 1/sqrt(std2); P = exp(score*T) (row denom via ACT accum)
  PT = P*T (row sum A via fused reduce); PT^T via PE transpose
  pv = PT @ [v_s | 1 | m_s]  -> out = (pv + A*v_r - (m_r A + MS)) / denom
  outputs: W'_proj @ out^T + b', W_r @ v_rT + r_b, W_s @ v_sT_own + s_b
"""
import os
import sys
from contextlib import ExitStack

import numpy as np

sys.path.insert(0, "/opt/trn_rl_repo")

import ml_dtypes  # noqa: E402
import concourse.bass as bass  # noqa: E402
import concourse.tile as tile  # noqa: E402
from concourse import bacc  # noqa: E402
from concourse import mybir  # noqa: E402
from concourse.bass_utils import run_bass_kernel_spmd  # noqa: E402

N, B, C = 3072, 4, 128
NO = N // 2          # own receiver rows per core
ITI = NO // 128      # 12 i-tiles
JCH = N // 512       # 6 j-chunks
JT = N // 128        # 24 j-tiles
EPS = 1e-5
SCALE = C ** -0.5
MASKM = 60.0         # masked-score bias: exp((score-M)*T) <= ~1e-14, ACT-range safe

F32 = mybir.dt.float32
BF16 = mybir.dt.bfloat16
U8 = mybir.dt.uint8
AF = mybir.ActivationFunctionType
ALU = mybir.AluOpType
AX = mybir.AxisListType

_CACHE = {}


def _build_program():
    nc = bacc.Bacc("TRN2", target_bir_lowering=False, debug=False, num_devices=8)

    def din(name, shape, dtype=F32):
        return nc.dram_tensor(name, list(shape), dtype, kind="ExternalInput").ap()

    def dout(name, shape, dtype=F32):
        return nc.dram_tensor(name, list(shape), dtype, kind="ExternalOutput").ap()

    xT_d = din("xT", [C, N])
    xTo_d = din("xTo", [C, NO])
    sendT_d = din("sendT", [C, N])
    sendTo_d = din("sendTo", [C, NO])
    res_sT_d = din("res_sT", [C, N])
    res_sTo_d = din("res_sTo", [C, NO])
    recvTo_d = din("recvTo", [C, NO])
    res_rTo_d = din("res_rTo", [C, NO])
    mask_d = din("mask", [NO, N], U8)
    w_send_d = din("w_send", [C, C])
    w_mem_d = din("w_mem", [C, C])
    w_recv_d = din("w_recv", [C, C])
    w_qs_d = din("w_qs", [C, C])
    w_proj_d = din("w_proj", [C, C])
    w_r_d = din("w_r", [C, C])
    w_s_d = din("w_s", [C, C])
    bp_d = din("bp", [C, 1])
    br_d = din("br", [C, 1])
    bs_d = din("bs", [C, 1])
    idf_d = din("idf", [C, C])
    idb_d = din("idb", [C, C], BF16)

    scr_mr_d = nc.dram_tensor("scr_mr", [1, NO], F32).ap()
    scr_ue_d = nc.dram_tensor("scr_ue", [1, NO], F32).ap()
    scr_ms_d = nc.dram_tensor("scr_ms", [1, N], F32).ap()
    outT_d = dout("outT", [C, NO])
    vr2T_d = dout("vr2T", [C, NO])
    vs2T_d = dout("vs2T", [C, NO])

    with tile.TileContext(nc) as tc, ExitStack() as ctx:
        const = ctx.enter_context(tc.tile_pool(name="const", bufs=1))
        per = ctx.enter_context(tc.tile_pool(name="per", bufs=1))
        stat = ctx.enter_context(tc.tile_pool(name="stat", bufs=1))
        stmp = ctx.enter_context(tc.tile_pool(name="stmp", bufs=2))
        ck = ctx.enter_context(tc.tile_pool(name="ck", bufs=2))
        strm = ctx.enter_context(tc.tile_pool(name="strm", bufs=6))
        mpool = ctx.enter_context(tc.tile_pool(name="mask", bufs=2))
        ps_mm = ctx.enter_context(tc.tile_pool(name="ps_mm", bufs=4, space="PSUM"))
        ps_tp = ctx.enter_context(tc.tile_pool(name="ps_tp", bufs=2, space="PSUM"))
        ps_pv = ctx.enter_context(tc.tile_pool(name="ps_pv", bufs=1, space="PSUM"))

        # ---------------- constants ----------------
        def cload(name, d_ap, shape, dtype=F32):
            t = const.tile(shape, dtype, tag=name)
            nc.sync.dma_start(t[:], d_ap)
            return t

        w_send = cload("w_send", w_send_d, [C, C])
        w_mem = cload("w_mem", w_mem_d, [C, C])
        w_recv = cload("w_recv", w_recv_d, [C, C])
        w_qs = cload("w_qs", w_qs_d, [C, C])
        w_proj = cload("w_proj", w_proj_d, [C, C])
        w_r = cload("w_r", w_r_d, [C, C])
        w_s = cload("w_s", w_s_d, [C, C])
        bp = cload("bp", bp_d, [C, 1])
        br_c = cload("br_c", br_d, [C, 1])
        bs_c = cload("bs_c", bs_d, [C, 1])
        idf = cload("idf", idf_d, [C, C])
        idb = cload("idb", idb_d, [C, C], BF16)

        oneD = const.tile([C, 1], F32)
        nc.gpsimd.memset(oneD[:], 1.0 / C)
        one = const.tile([C, 1], F32)
        nc.gpsimd.memset(one[:], 1.0)
        eps1 = const.tile([1, 1], F32)
        nc.gpsimd.memset(eps1[:], EPS)

        # persistent tensors
        v_sT = per.tile([C, N], F32)
        v_sTo = per.tile([C, NO], F32)
        v_rT = per.tile([C, NO], F32)
        qT = per.tile([C, NO], F32)
        vr_s = per.tile([C, NO], F32)          # v_rT * 2/C (cross lhsT)
        v_r_nat = per.tile([C, ITI * C], F32)
        v_s_aug = per.tile([C, JT * (C + 2)], BF16)
        outT_pre = per.tile([C, NO], F32)

        aug1_rhs = stat.tile([2, N], F32)      # row0 = m_s, row1 = w_j
        aug2_rhs = stat.tile([2, N], F32)      # row0 = -m_s, row1 = 1
        aug1_lhsT = stat.tile([2, NO], F32)    # row0 = -2 m_r, row1 = 1
        aug2_lhsT = stat.tile([2, NO], F32)    # row0 = sumq, row1 = alpha
        m_r_row = stat.tile([1, NO], F32)
        u_eps_row = stat.tile([1, NO], F32)
        w_row = stat.tile([1, N], F32)
        alpha_row = stat.tile([1, NO], F32)
        m_r_cols = stat.tile([C, ITI], F32)
        u_eps_cols = stat.tile([C, ITI], F32)
        m_s_cols = stat.tile([C, JT], F32)

        nc.vector.memset(aug2_rhs[:, :], 1.0)  # row0 overwritten with -m_s below
        nc.vector.memset(aug1_lhsT[:, :], 1.0)  # row0 overwritten with -2*m_r below

        def stream(d_ap, sl):
            t = strm.tile([C, 512], F32, tag="instream")
            nc.sync.dma_start(t[:], d_ap[:, sl])
            return t

        # -------- phase 1: value tensors (inputs streamed chunk-wise) --------
        for jc in range(JCH):
            sl = bass.ts(jc, 512)
            xc = stream(xT_d, sl)
            sc = stream(sendT_d, sl)
            rc = stream(res_sT_d, sl)
            ps = ps_mm.tile([C, 512], F32, tag="mm")
            nc.tensor.matmul(ps[:], w_send[:], xc[:], start=True, stop=False)
            nc.tensor.matmul(ps[:], w_mem[:], sc[:], start=False, stop=True)
            nc.vector.tensor_tensor(out=v_sT[:, sl], in0=ps[:], in1=rc[:], op=ALU.add)
        for c3 in range(NO // 512):
            sl = bass.ts(c3, 512)
            xc = stream(xTo_d, sl)
            sc = stream(sendTo_d, sl)
            rc = stream(res_sTo_d, sl)
            ps = ps_mm.tile([C, 512], F32, tag="mm")
            nc.tensor.matmul(ps[:], w_send[:], xc[:], start=True, stop=False)
            nc.tensor.matmul(ps[:], w_mem[:], sc[:], start=False, stop=True)
            nc.vector.tensor_tensor(out=v_sTo[:, sl], in0=ps[:], in1=rc[:], op=ALU.add)
            rcv = stream(recvTo_d, sl)
            rrc = stream(res_rTo_d, sl)
            ps2 = ps_mm.tile([C, 512], F32, tag="mm")
            nc.tensor.matmul(ps2[:], w_recv[:], xc[:], start=True, stop=False)
            nc.tensor.matmul(ps2[:], w_mem[:], rcv[:], start=False, stop=True)
            nc.vector.tensor_tensor(out=v_rT[:, sl], in0=ps2[:], in1=rrc[:], op=ALU.add)
            ps3 = ps_mm.tile([C, 512], F32, tag="mm")
            nc.tensor.matmul(ps3[:], w_qs[:], xc[:], start=True, stop=True)
            nc.scalar.copy(qT[:, sl], ps3[:])

        # -------- phase 2: stats --------
        # sender-side stats into aug rows
        for jc in range(JCH):
            sl = bass.ts(jc, 512)
            psm = ps_pv.tile([1, 512], F32, tag="row")
            nc.tensor.matmul(psm[:], oneD[:], v_sT[:, sl], start=True, stop=True)
            nc.scalar.copy(aug1_rhs[0:1, sl], psm[:])          # m_s
            sqc = ck.tile([C, 512], F32, tag="sqc")
            nc.scalar.activation(sqc[:], v_sT[:, sl], AF.Square)
            psq = ps_pv.tile([1, 512], F32, tag="row")
            nc.tensor.matmul(psq[:], oneD[:], sqc[:], start=True, stop=True)
            trow = stmp.tile([1, 512], F32, tag="trow")
            nc.vector.tensor_tensor(out=trow[:], in0=aug1_rhs[0:1, sl], in1=aug1_rhs[0:1, sl], op=ALU.mult)
            nc.vector.tensor_tensor(out=w_row[:, sl], in0=psq[:], in1=trow[:], op=ALU.subtract)  # w_j
            nc.scalar.mul(aug2_rhs[0:1, sl], aug1_rhs[0:1, sl], -1.0)  # -m_s

        # receiver-side stats
        for c3 in range(NO // 512):
            sl = bass.ts(c3, 512)
            psm = ps_pv.tile([1, 512], F32, tag="row")
            nc.tensor.matmul(psm[:], oneD[:], v_rT[:, sl], start=True, stop=True)
            nc.scalar.copy(m_r_row[:, sl], psm[:])
            sqc = ck.tile([C, 512], F32, tag="sqc")
            nc.scalar.activation(sqc[:], v_rT[:, sl], AF.Square)
            psq = ps_pv.tile([1, 512], F32, tag="row")
            nc.tensor.matmul(psq[:], oneD[:], sqc[:], start=True, stop=True)
            nc.scalar.activation(u_eps_row[:, sl], psq[:], AF.Identity, bias=eps1[:])
            trow = stmp.tile([1, 512], F32, tag="trow")
            nc.vector.tensor_tensor(out=trow[:], in0=m_r_row[:, sl], in1=m_r_row[:, sl], op=ALU.mult)
            nc.vector.tensor_tensor(out=u_eps_row[:, sl], in0=u_eps_row[:, sl], in1=trow[:], op=ALU.subtract)
            # sumq
            pss = ps_pv.tile([1, 512], F32, tag="row")
            nc.tensor.matmul(pss[:], one[:], qT[:, sl], start=True, stop=True)
            nc.scalar.copy(aug2_lhsT[0:1, sl], pss[:])
            # alpha = sum(q*v_r) - sumq*m_r
            qv = ck.tile([C, 512], F32, tag="sqc")
            nc.vector.tensor_tensor(out=qv[:], in0=qT[:, sl], in1=v_rT[:, sl], op=ALU.mult)
            psa = ps_pv.tile([1, 512], F32, tag="row")
            nc.tensor.matmul(psa[:], one[:], qv[:], start=True, stop=True)
            trow2 = stmp.tile([1, 512], F32, tag="trow")
            nc.vector.tensor_tensor(out=trow2[:], in0=aug2_lhsT[0:1, sl], in1=m_r_row[:, sl], op=ALU.mult)
            nc.vector.tensor_tensor(out=alpha_row[:, sl], in0=psa[:], in1=trow2[:], op=ALU.subtract)

        nc.scalar.mul(aug1_lhsT[0:1, :], m_r_row[:], -2.0)
        nc.sync.dma_start(aug1_rhs[1:2, :], w_row[:])
        nc.sync.dma_start(aug2_lhsT[1:2, :], alpha_row[:])

        # row -> column layouts via DRAM round-trip (element (p,t) = row[t*128+p])
        nc.sync.dma_start(scr_mr_d, m_r_row[:])
        nc.sync.dma_start(m_r_cols[:], scr_mr_d.rearrange("o (t p) -> (o p) t", p=128))
        nc.sync.dma_start(scr_ue_d, u_eps_row[:])
        nc.sync.dma_start(u_eps_cols[:], scr_ue_d.rearrange("o (t p) -> (o p) t", p=128))
        nc.sync.dma_start(scr_ms_d, aug1_rhs[0:1, :])
        nc.sync.dma_start(m_s_cols[:], scr_ms_d.rearrange("o (t p) -> (o p) t", p=128))

        # v_s natural (bf16, augmented) + v_r natural + scaled v_r
        v_s_aug_r = v_s_aug[:].rearrange("p (t c) -> p t c", c=C + 2)
        for g in range(JT // 4):
            pst = ps_tp.tile([C, 512], F32, tag="tp")
            for t in range(4):
                jt = g * 4 + t
                nc.tensor.transpose(pst[:, bass.ts(t, 128)], v_sT[:, bass.ts(jt, 128)], idf[:])
            src = pst[:].rearrange("p (t c) -> p t c", c=C)
            nc.scalar.copy(v_s_aug_r[:, g * 4:(g + 1) * 4, 0:C], src)
        nc.gpsimd.memset(v_s_aug_r[:, :, C:C + 1], 1.0)
        m_s_cols_r = m_s_cols[:].rearrange("p (t o) -> p t o", o=1)
        nc.scalar.copy(v_s_aug_r[:, :, C + 1:C + 2], m_s_cols_r)

        for g in range(ITI // 4):
            pst = ps_tp.tile([C, 512], F32, tag="tp")
            for t in range(4):
                it = g * 4 + t
                nc.tensor.transpose(pst[:, bass.ts(t, 128)], v_rT[:, bass.ts(it, 128)], idf[:])
            nc.scalar.copy(v_r_nat[:, bass.ts(g, 512)], pst[:])
        nc.scalar.mul(vr_s[:], v_rT[:], 2.0 / C)

        # -------- phase 3: main attention loop --------
        for it in range(ITI):
            isl = bass.ts(it, 128)
            mk8 = mpool.tile([C, N], U8, tag="mk8")
            nc.sync.dma_start(mk8[:], mask_d[bass.ts(it, 128), :])
            mkb = mpool.tile([C, N], BF16, tag="mkb")
            nc.gpsimd.tensor_scalar_mul(mkb[:], mk8[:], -MASKM)

            den_part = stmp.tile([C, 8], F32, tag="den_part")
            pv = ps_pv.tile([C, C + 2], F32)

            for jc in range(JCH):
                jsl = bass.ts(jc, 512)
                ps_v = ps_mm.tile([C, 512], F32, tag="mm")
                nc.tensor.matmul(ps_v[:], vr_s[:, isl], v_sT[:, jsl], start=True, stop=False)
                nc.tensor.matmul(ps_v[:], aug1_lhsT[:, isl], aug1_rhs[:, jsl], start=False, stop=True)
                ps_s = ps_mm.tile([C, 512], F32, tag="mm")
                nc.tensor.matmul(ps_s[:], qT[:, isl], v_sT[:, jsl], start=True, stop=False)
                nc.tensor.matmul(ps_s[:], aug2_lhsT[:, isl], aug2_rhs[:, jsl], start=False, stop=False)
                nc.tensor.matmul(ps_s[:], idb[:], mkb[:, jsl], start=False, stop=True)

                stdc = ck.tile([C, 512], F32, tag="stdc")
                nc.scalar.activation(stdc[:], ps_v[:], AF.Sqrt, bias=u_eps_cols[:, it:it + 1])
                tcc = ck.tile([C, 512], F32, tag="tcc")
                nc.vector.reciprocal_approx_fast(out=tcc[:], in_=stdc[:])
                uc = ck.tile([C, 512], F32, tag="uc")
                nc.vector.tensor_tensor(out=uc[:], in0=ps_s[:], in1=tcc[:], op=ALU.mult)
                pc = ck.tile([C, 512], F32, tag="pc")
                nc.scalar.activation(pc[:], uc[:], AF.Exp, accum_out=den_part[:, jc:jc + 1])
                ptc = ck.tile([C, 512], F32, tag="ptc")
                nc.vector.tensor_tensor(out=ptc[:], in0=pc[:], in1=tcc[:], op=ALU.mult)

                pst = ps_tp.tile([C, 512], F32, tag="tp")
                for t in range(4):
                    nc.tensor.transpose(pst[:, bass.ts(t, 128)], ptc[:, bass.ts(t, 128)], idf[:])
                pttc = ck.tile([C, 512], BF16, tag="pttc")
                nc.scalar.copy(pttc[:], pst[:])
                for t in range(4):
                    jt = jc * 4 + t
                    nc.tensor.matmul(
                        pv[:], pttc[:, bass.ts(t, 128)], v_s_aug_r[:, jt, :],
                        start=(jc == 0 and t == 0), stop=(jc == JCH - 1 and t == 3))

            den = stmp.tile([C, 1], F32, tag="den")
            nc.vector.tensor_reduce(den[:], den_part[:, 0:JCH], axis=AX.X, op=ALU.add)
            rcol = stmp.tile([C, 1], F32, tag="rcol")
            nc.vector.reciprocal(rcol[:], den[:])
            ams = stmp.tile([C, 2], F32, tag="ams")
            nc.scalar.copy(ams[:], pv[:, C:C + 2])
            t1 = stmp.tile([C, 1], F32, tag="t1")
            nc.vector.scalar_tensor_tensor(
                out=t1[:], in0=ams[:, 0:1], scalar=m_r_cols[:, it:it + 1], in1=ams[:, 1:2],
                op0=ALU.mult, op1=ALU.add)
            brr = stmp.tile([C, 1], F32, tag="brr")
            nc.vector.scalar_tensor_tensor(
                out=brr[:], in0=t1[:], scalar=-1.0, in1=rcol[:], op0=ALU.mult, op1=ALU.mult)
            x1 = stmp.tile([C, C], F32, tag="x1")
            nc.vector.scalar_tensor_tensor(
                out=x1[:], in0=v_r_nat[:, isl], scalar=ams[:, 0:1], in1=pv[:, 0:C],
                op0=ALU.mult, op1=ALU.add)
            x2 = stmp.tile([C, C], F32, tag="x2")
            nc.scalar.activation(x2[:], x1[:], AF.Identity, bias=brr[:], scale=rcol[:])
            pso = ps_tp.tile([C, C], F32, tag="tp")
            nc.tensor.transpose(pso[:], x2[:], idf[:])
            nc.scalar.copy(outT_pre[:, isl], pso[:])

        # -------- phase 4: output projections --------
        for w, bias_col, rhs, out_d in (
            (w_proj, bp, outT_pre, outT_d),
            (w_r, br_c, v_rT, vr2T_d),
            (w_s, bs_c, v_sTo, vs2T_d),
        ):
            for c3 in range(NO // 512):
                sl = bass.ts(c3, 512)
                ps = ps_mm.tile([C, 512], F32, tag="mm")
                nc.tensor.matmul(ps[:], w[:], rhs[:, sl], start=True, stop=True)
                ob = stmp.tile([C, 512], F32, tag="ob")
                nc.scalar.activation(ob[:], ps[:], AF.Identity, bias=bias_col[:])
                nc.sync.dma_start(out_d[:, sl], ob[:])

    nc.compile()
    return nc


def _host_prep(inputs):
    """Returns (in_maps list of 8 dicts, misc)"""
    f32 = np.float32
    x = np.ascontiguousarray(np.asarray(inputs["x"], f32))
    recv = np.asarray(inputs["receiver_val_res"], f32)
    send = np.asarray(inputs["sender_val_res"], f32)
    res_r = np.asarray(inputs["residual_receiver"], f32)
    res_s = np.asarray(inputs["residual_sender"], f32)
    mask = np.asarray(inputs["attn_mask"])
    ra = np.asarray(inputs["relation_attn"], f32)
    q_w = np.asarray(inputs["q_w"], f32)
    proj_w = np.asarray(inputs["proj_w"], f32)
    proj_b = np.asarray(inputs["proj_b"], f32)
    r_w = np.asarray(inputs["r_w"], f32)
    r_b = np.asarray(inputs["r_b"], f32)
    s_w = np.asarray(inputs["s_w"], f32)
    s_b = np.asarray(inputs["s_b"], f32)
    n_weight = np.asarray(inputs["n_weight"], f32)
    n_bias = np.asarray(inputs["n_bias"], f32)

    mem_w, recv_w, send_w = ra[:, :C], ra[:, C:2 * C], ra[:, 2 * C:]
    w_proj_eff = proj_w * n_weight[None, :]
    b_proj_eff = proj_w @ n_bias + proj_b

    cc = np.ascontiguousarray
    weights = {
        "w_send": cc(send_w.T), "w_mem": cc(mem_w.T), "w_recv": cc(recv_w.T),
        "w_qs": cc(q_w.T * SCALE), "w_proj": cc(w_proj_eff.T),
        "w_r": cc(r_w.T), "w_s": cc(s_w.T),
        "bp": cc(b_proj_eff[:, None]), "br": cc(r_b[:, None]), "bs": cc(s_b[:, None]),
        "idf": cc(np.eye(C, dtype=f32)),
        "idb": cc(np.eye(C).astype(ml_dtypes.bfloat16)),
    }

    in_maps = []
    for core in range(8):
        b, half = core // 2, core % 2
        i0, i1 = half * NO, (half + 1) * NO
        xb = cc(x[:, b, :].T)                      # [C, N]
        sb = cc(send[:, b, :].T)
        rsb = cc(res_s[:, b, :].T)
        m = {
            "xT": xb, "xTo": cc(xb[:, i0:i1]),
            "sendT": sb, "sendTo": cc(sb[:, i0:i1]),
            "res_sT": rsb, "res_sTo": cc(rsb[:, i0:i1]),
            "recvTo": cc(recv[i0:i1, b, :].T),
            "res_rTo": cc(res_r[i0:i1, b, :].T),
            "mask": cc(mask[b, 0, i0:i1, :].astype(np.uint8)),
        }
        m.update(weights)
        in_maps.append(m)
    return in_maps


def kernel(**inputs):
    if "nc" not in _CACHE:
        _CACHE["nc"] = _build_program()
    nc = _CACHE["nc"]
    in_maps = _host_prep(inputs)
    res = run_bass_kernel_spmd(nc, in_maps, core_ids=list(range(8)))
    out = np.zeros((N, B, C), np.float32)
    vr2 = np.zeros((N, B, C), np.float32)
    vs2 = np.zeros((N, B, C), np.float32)
    for core in range(8):
        b, half = core // 2, core % 2
        i0, i1 = half * NO, (half + 1) * NO
        r = res.results[core]
        out[i0:i1, b, :] = r["outT"].T
        vr2[i0:i1, b, :] = r["vr2T"].T
        vs2[i0:i1, b, :] = r["vs2T"].T
    return out, vr2, vs2



# revision 2
# speedup vs baseline: 1.2299x; 1.2299x over previous
"""Trainium2 Bass kernel for nn_AttentionTIE (TIE-style edge-LayerNorm attention).

Sharding: 8 cores = (batch b = core//2) x (receiver-row half = core%2).
Each core computes the full v_sender for its batch, attention for its 1536
receiver rows, and the three projected outputs for those rows.

v2 design (vs v1 baseline):
  - all big matmuls stream float32r (1 cyc/row at free>=256 vs 4 for f32)
  - sender values centered (c_s = v_s - m_s): the q.v_s and cross matmuls
    then absorb the m_s rank-1 terms, killing two aug matmul passes
  - T = 1/std computed as Exp(-0.5*Ln(std2)): Ln and Exp share one ACT
    table ('natural_log_exp_and_others') so the main loop never reloads
    activation tables (v1 lost 124us to Sqrt<->Exp table thrash)
  - P pipeline in bf16: tcc/pc/ptc bf16, PE transpose in bf16, and the
    PT^T PSUM->SBUF move rides the DMA engine instead of ACT

Algorithm per core (all shapes [partition, free]):
  v_sT  = W_send @ xT + W_mem @ sendT + res_sT            [C, N]
  c_sT  = v_sT - 1 (x) m_s                                [C, N]
  v_rT  = W_recv @ xT_own + W_mem @ recvT_own + res_rT    [C, No]
  qT    = (W_q*scale) @ xT_own                            [C, No]
  std2[i,j] = (u_i+eps) + w_j + (2/D) v_r . c_s           (w_j via rank-1 aug)
  score[i,j] = q . c_s - M*mask ; uc = (score+alpha_i)*T
  T = Exp(-0.5 Ln(std2+bias)); P = Exp(uc) (row denom via ACT accum)
  PT = P*T; PT^T via PE transpose (bf16); pv = PT @ [v_s | 1 | m_s]
  out = (pv + A*v_r - (m_r A + MS)) / denom
  outputs: W'_proj @ out^T + b', W_r @ v_rT + r_b, W_s @ v_sT_own + s_b
"""
import os
import sys
from contextlib import ExitStack

import numpy as np

sys.path.insert(0, "/opt/trn_rl_repo")

import ml_dtypes  # noqa: E402
import concourse.bass as bass  # noqa: E402
import concourse.tile as tile  # noqa: E402
from concourse import bacc  # noqa: E402
from concourse import mybir  # noqa: E402
from concourse.bass_utils import run_bass_kernel_spmd  # noqa: E402

N, B, C = 3072, 4, 128
NO = N // 2          # own receiver rows per core
ITI = NO // 128      # 12 i-tiles
JCH = N // 512       # 6 j-chunks
JT = N // 128        # 24 j-tiles
EPS = 1e-5
SCALE = C ** -0.5
MASKM = 60.0         # masked-score bias: exp((score-M)*T) <= ~1e-14, ACT-range safe

F32 = mybir.dt.float32
F32R = mybir.dt.float32r
BF16 = mybir.dt.bfloat16
U8 = mybir.dt.uint8
AF = mybir.ActivationFunctionType
ALU = mybir.AluOpType
AX = mybir.AxisListType

_CACHE = {}


def _build_program():
    nc = bacc.Bacc("TRN2", target_bir_lowering=False, debug=False, num_devices=8)

    def din(name, shape, dtype=F32):
        return nc.dram_tensor(name, list(shape), dtype, kind="ExternalInput").ap()

    def dout(name, shape, dtype=F32):
        return nc.dram_tensor(name, list(shape), dtype, kind="ExternalOutput").ap()

    xT_d = din("xT", [C, N])
    xTo_d = din("xTo", [C, NO])
    sendT_d = din("sendT", [C, N])
    sendTo_d = din("sendTo", [C, NO])
    res_sT_d = din("res_sT", [C, N])
    res_sTo_d = din("res_sTo", [C, NO])
    recvTo_d = din("recvTo", [C, NO])
    res_rTo_d = din("res_rTo", [C, NO])
    mask_d = din("mask", [NO, N], U8)
    w_send_d = din("w_send", [C, C])
    w_mem_d = din("w_mem", [C, C])
    w_recv_d = din("w_recv", [C, C])
    w_qs_d = din("w_qs", [C, C])
    w_proj_d = din("w_proj", [C, C])
    w_r_d = din("w_r", [C, C])
    w_s_d = din("w_s", [C, C])
    bp_d = din("bp", [C, 1])
    br_d = din("br", [C, 1])
    bs_d = din("bs", [C, 1])
    idf_d = din("idf", [C, C])
    idb_d = din("idb", [C, C], BF16)
    oneD_d = din("oneD", [C, 1])
    one_d = din("one", [C, 1])
    ones_row_d = din("ones_row", [1, NO])

    scr_mr_d = nc.dram_tensor("scr_mr", [1, NO], F32).ap()
    scr_ue_d = nc.dram_tensor("scr_ue", [1, NO], F32).ap()
    scr_al_d = nc.dram_tensor("scr_al", [1, NO], F32).ap()
    scr_ms_d = nc.dram_tensor("scr_ms", [1, N], F32).ap()
    outT_d = dout("outT", [C, NO])
    vr2T_d = dout("vr2T", [C, NO])
    vs2T_d = dout("vs2T", [C, NO])

    def r32(ap):
        return ap.bitcast(F32R)

    def f32v(ap):
        return ap.bitcast(F32)

    with tile.TileContext(nc) as tc, ExitStack() as ctx:
        const = ctx.enter_context(tc.tile_pool(name="const", bufs=1))
        per = ctx.enter_context(tc.tile_pool(name="per", bufs=1))
        stat = ctx.enter_context(tc.tile_pool(name="stat", bufs=1))
        stmp = ctx.enter_context(tc.tile_pool(name="stmp", bufs=2))
        ck = ctx.enter_context(tc.tile_pool(name="ck", bufs=3))
        strm = ctx.enter_context(tc.tile_pool(name="strm", bufs=6))
        mpool = ctx.enter_context(tc.tile_pool(name="mask", bufs=2))
        ps_mm = ctx.enter_context(tc.tile_pool(name="ps_mm", bufs=4, space="PSUM"))
        ps_tp = ctx.enter_context(tc.tile_pool(name="ps_tp", bufs=2, space="PSUM"))
        ps_pv = ctx.enter_context(tc.tile_pool(name="ps_pv", bufs=1, space="PSUM"))

        # Pre-place a load of the 'natural_log_exp_and_others' ACT table: it
        # serves every activation this kernel uses (Ln, Exp, Square, Copy,
        # Identity), so the compile-time table-load pass inserts no further
        # loads (the greedy default would thrash Ln<->Exp tables every chunk).
        nc.scalar.add_instruction(mybir.InstLoadActFuncSet(
            name=nc.get_next_instruction_name(), engine=mybir.EngineType.Activation,
            act_func_set_id=6, ins=[], outs=[]))

        # ---------------- constants ----------------
        def cload(name, d_ap, shape, dtype=F32):
            t = const.tile(shape, dtype, tag=name, name=name)
            nc.sync.dma_start(t[:], d_ap)
            return t

        def cload_r(name, d_ap, shape):
            t = const.tile(shape, F32R, tag=name, name=name)
            nc.sync.dma_start(t[:], d_ap.bitcast(F32R))
            return t

        w_send = cload_r("w_send", w_send_d, [C, C])
        w_mem = cload_r("w_mem", w_mem_d, [C, C])
        w_recv = cload_r("w_recv", w_recv_d, [C, C])
        w_qs = cload_r("w_qs", w_qs_d, [C, C])
        w_proj = cload_r("w_proj", w_proj_d, [C, C])
        w_r = cload_r("w_r", w_r_d, [C, C])
        w_s = cload_r("w_s", w_s_d, [C, C])
        bp = cload("bp", bp_d, [C, 1])
        br_c = cload("br_c", br_d, [C, 1])
        bs_c = cload("bs_c", bs_d, [C, 1])
        idf = cload_r("idf", idf_d, [C, C])
        idb = cload("idb", idb_d, [C, C], BF16)

        oneD = const.tile([C, 1], F32R)
        nc.sync.dma_start(oneD[:], oneD_d.bitcast(F32R))
        one = const.tile([C, 1], F32R)
        nc.sync.dma_start(one[:], one_d.bitcast(F32R))
        ones_row = const.tile([1, NO], F32R)
        nc.sync.dma_start(ones_row[:], ones_row_d.bitcast(F32R))
        eps1 = const.tile([1, 1], F32)
        nc.gpsimd.memset(eps1[:], EPS)

        # persistent tensors
        v_sT = per.tile([C, N], F32R)
        c_sT = per.tile([C, N], F32R)
        v_sTo = per.tile([C, NO], F32R)
        v_rT = per.tile([C, NO], F32R)
        qT = per.tile([C, NO], F32R)
        vr_s = per.tile([C, NO], F32R)         # v_rT * 2/C (cross lhsT)
        v_r_nat = per.tile([C, ITI * C], F32)
        v_s_bf = per.tile([C, N], BF16)
        v_s_aug = per.tile([C, JT * (C + 2)], BF16)
        outT_pre = per.tile([C, NO], F32R)

        m_r_row = stat.tile([1, NO], F32)
        u_eps_row = stat.tile([1, NO], F32)
        w_row = stat.tile([1, N], F32R)        # var_s per sender (rank-1 aug rhs)
        m_s_row = stat.tile([1, N], F32R)
        alpha_row = stat.tile([1, NO], F32)
        sumq_row = stat.tile([1, NO], F32)
        m_r_cols = stat.tile([C, ITI], F32)
        u_eps_cols = stat.tile([C, ITI], F32)
        al_cols = stat.tile([C, ITI], F32)
        m_s_cols = stat.tile([C, JT], F32)

        def stream(d_ap, sl, dt=F32):
            t = strm.tile([C, 512], dt, tag="instream", name="instream")
            src = d_ap[:, sl]
            nc.sync.dma_start(t[:], src.bitcast(dt) if dt is F32R else src)
            return t

        # -------- phase 1: value tensors (inputs streamed chunk-wise) --------
        for jc in range(JCH):
            sl = bass.ts(jc, 512)
            xc = stream(xT_d, sl, F32R)
            sc = stream(sendT_d, sl, F32R)
            rc = stream(res_sT_d, sl)
            ps = ps_mm.tile([C, 512], F32, tag="mm", name="mm")
            nc.tensor.matmul(ps[:], w_send[:], xc[:], start=True, stop=False)
            nc.tensor.matmul(ps[:], w_mem[:], sc[:], start=False, stop=True)
            nc.vector.tensor_tensor(out=v_sT[:, sl], in0=ps[:], in1=rc[:], op=ALU.add)
        for c3 in range(NO // 512):
            sl = bass.ts(c3, 512)
            xc = stream(xTo_d, sl, F32R)
            sc = stream(sendTo_d, sl, F32R)
            rc = stream(res_sTo_d, sl)
            ps = ps_mm.tile([C, 512], F32, tag="mm", name="mm")
            nc.tensor.matmul(ps[:], w_send[:], xc[:], start=True, stop=False)
            nc.tensor.matmul(ps[:], w_mem[:], sc[:], start=False, stop=True)
            nc.vector.tensor_tensor(out=v_sTo[:, sl], in0=ps[:], in1=rc[:], op=ALU.add)
            rcv = stream(recvTo_d, sl, F32R)
            rrc = stream(res_rTo_d, sl)
            ps2 = ps_mm.tile([C, 512], F32, tag="mm", name="mm")
            nc.tensor.matmul(ps2[:], w_recv[:], xc[:], start=True, stop=False)
            nc.tensor.matmul(ps2[:], w_mem[:], rcv[:], start=False, stop=True)
            nc.vector.tensor_tensor(out=v_rT[:, sl], in0=ps2[:], in1=rrc[:], op=ALU.add)
            ps3 = ps_mm.tile([C, 512], F32, tag="mm", name="mm")
            nc.tensor.matmul(ps3[:], w_qs[:], xc[:], start=True, stop=True)
            nc.scalar.copy(qT[:, sl], ps3[:])

        # -------- phase 2: stats --------
        # sender-side stats: m_s, w = var_s
        for jc in range(JCH):
            sl = bass.ts(jc, 512)
            psm = ps_pv.tile([1, 512], F32, tag="row", name="row")
            nc.tensor.matmul(psm[:], oneD[:], v_sT[:, sl], start=True, stop=True)
            nc.scalar.copy(m_s_row[:, sl], psm[:])
            sqc = ck.tile([C, 512], F32R, tag="sqc", name="sqc")
            nc.scalar.activation(sqc[:], f32v(v_sT[:, sl]), AF.Square)
            psq = ps_pv.tile([1, 512], F32, tag="row", name="row")
            nc.tensor.matmul(psq[:], oneD[:], sqc[:], start=True, stop=True)
            trow = stmp.tile([1, 512], F32, tag="trow", name="trow")
            nc.vector.tensor_tensor(out=trow[:], in0=f32v(m_s_row[0:1, sl]), in1=f32v(m_s_row[0:1, sl]), op=ALU.mult)
            nc.vector.tensor_tensor(out=w_row[:, sl], in0=psq[:], in1=trow[:], op=ALU.subtract)

        # centered senders: c_sT = v_sT - 1 (x) m_s ; also bf16 copy of v_sT
        for jc in range(JCH):
            sl = bass.ts(jc, 512)
            psb = ps_mm.tile([C, 512], F32, tag="mm", name="mm")
            nc.tensor.matmul(psb[:], ones_row[0:1, 0:C], m_s_row[0:1, sl], start=True, stop=True)
            nc.vector.tensor_tensor(out=c_sT[:, sl], in0=f32v(v_sT[:, sl]), in1=psb[:], op=ALU.subtract)
            nc.scalar.copy(v_s_bf[:, sl], f32v(v_sT[:, sl]))

        # receiver-side stats
        for c3 in range(NO // 512):
            sl = bass.ts(c3, 512)
            psm = ps_pv.tile([1, 512], F32, tag="row", name="row")
            nc.tensor.matmul(psm[:], oneD[:], v_rT[:, sl], start=True, stop=True)
            nc.scalar.copy(m_r_row[:, sl], psm[:])
            sqc = ck.tile([C, 512], F32R, tag="sqc", name="sqc")
            nc.scalar.activation(sqc[:], f32v(v_rT[:, sl]), AF.Square)
            psq = ps_pv.tile([1, 512], F32, tag="row", name="row")
            nc.tensor.matmul(psq[:], oneD[:], sqc[:], start=True, stop=True)
            nc.scalar.activation(u_eps_row[:, sl], psq[:], AF.Identity, bias=eps1[:])
            trow = stmp.tile([1, 512], F32, tag="trow", name="trow")
            nc.vector.tensor_tensor(out=trow[:], in0=m_r_row[:, sl], in1=m_r_row[:, sl], op=ALU.mult)
            nc.vector.tensor_tensor(out=u_eps_row[:, sl], in0=u_eps_row[:, sl], in1=trow[:], op=ALU.subtract)
            # sumq
            pss = ps_pv.tile([1, 512], F32, tag="row", name="row")
            nc.tensor.matmul(pss[:], one[:], qT[:, sl], start=True, stop=True)
            nc.scalar.copy(sumq_row[:, sl], pss[:])
            # alpha = sum(q*v_r) - sumq*m_r
            qv = ck.tile([C, 512], F32R, tag="sqc", name="sqc")
            nc.vector.tensor_tensor(out=qv[:], in0=f32v(qT[:, sl]), in1=f32v(v_rT[:, sl]), op=ALU.mult)
            psa = ps_pv.tile([1, 512], F32, tag="row", name="row")
            nc.tensor.matmul(psa[:], one[:], qv[:], start=True, stop=True)
            trow2 = stmp.tile([1, 512], F32, tag="trow", name="trow")
            nc.vector.tensor_tensor(out=trow2[:], in0=sumq_row[0:1, sl], in1=m_r_row[:, sl], op=ALU.mult)
            nc.vector.tensor_tensor(out=alpha_row[:, sl], in0=psa[:], in1=trow2[:], op=ALU.subtract)

        # row -> column layouts via DRAM round-trip (element (p,t) = row[t*128+p])
        nc.sync.dma_start(scr_mr_d, m_r_row[:])
        nc.sync.dma_start(m_r_cols[:], scr_mr_d.rearrange("o (t p) -> (o p) t", p=128))
        nc.sync.dma_start(scr_ue_d, u_eps_row[:])
        nc.sync.dma_start(u_eps_cols[:], scr_ue_d.rearrange("o (t p) -> (o p) t", p=128))
        nc.sync.dma_start(scr_al_d, alpha_row[:])
        nc.sync.dma_start(al_cols[:], scr_al_d.rearrange("o (t p) -> (o p) t", p=128))
        nc.sync.dma_start(scr_ms_d, f32v(m_s_row[:]))
        nc.sync.dma_start(m_s_cols[:], scr_ms_d.rearrange("o (t p) -> (o p) t", p=128))

        # v_s natural (bf16, augmented) + v_r natural + scaled v_r
        v_s_aug_r = v_s_aug[:].rearrange("p (t c) -> p t c", c=C + 2)
        for g in range(JT // 4):
            pst = ps_tp.tile([C, 512], BF16, tag="tp", name="tp")
            for t in range(4):
                jt = g * 4 + t
                nc.tensor.transpose(pst[:, bass.ts(t, 128)], v_s_bf[:, bass.ts(jt, 128)], idb[:])
            src = pst[:].rearrange("p (t c) -> p t c", c=C)
            nc.scalar.copy(v_s_aug_r[:, g * 4:(g + 1) * 4, 0:C], src)
        nc.gpsimd.memset(v_s_aug_r[:, :, C:C + 1], 1.0)
        m_s_cols_r = m_s_cols[:].rearrange("p (t o) -> p t o", o=1)
        nc.scalar.copy(v_s_aug_r[:, :, C + 1:C + 2], m_s_cols_r)

        for g in range(ITI // 4):
            pst = ps_tp.tile([C, 512], F32R, tag="tp", name="tp")
            for t in range(4):
                it = g * 4 + t
                nc.tensor.transpose(pst[:, bass.ts(t, 128)], v_rT[:, bass.ts(it, 128)], idf[:])
            nc.scalar.copy(v_r_nat[:, bass.ts(g, 512)], f32v(pst[:]))
        nc.scalar.mul(vr_s[:], f32v(v_rT[:]), 2.0 / C)

        # -------- phase 3: main attention loop --------
        for it in range(ITI):
            isl = bass.ts(it, 128)
            mk8 = mpool.tile([C, N], U8, tag="mk8", name="mk8")
            nc.sync.dma_start(mk8[:], mask_d[bass.ts(it, 128), :])
            mkb = mpool.tile([C, N], BF16, tag="mkb", name="mkb")
            nc.gpsimd.tensor_scalar_mul(mkb[:], mk8[:], -MASKM)

            den_part = stmp.tile([C, 8], F32, tag="den_part", name="den_part")
            pv = ps_pv.tile([C, C + 2], F32, tag="pv", name="pv")

            for jc in range(JCH):
                jsl = bass.ts(jc, 512)
                # std2 = (2/D) v_r.c_s  +  1 (x) w_j   (+ u_i + eps via Ln bias)
                ps_v = ps_mm.tile([C, 512], F32, tag="mm", name="mm")
                nc.tensor.matmul(ps_v[:], vr_s[:, isl], c_sT[:, jsl], start=True, stop=False)
                nc.tensor.matmul(ps_v[:], ones_row[0:1, isl], w_row[0:1, jsl], start=False, stop=True)
                # score = q.c_s - M*mask   (alpha added in the stt below)
                ps_s = ps_mm.tile([C, 512], F32, tag="mm", name="mm")
                nc.tensor.matmul(ps_s[:], qT[:, isl], c_sT[:, jsl], start=True, stop=False)
                nc.tensor.matmul(ps_s[:], idb[:], mkb[:, jsl], start=False, stop=True)

                # T = exp(-0.5 ln(std2)); Ln and Exp share one ACT table
                lc = ck.tile([C, 512], F32, tag="lc", name="lc")
                nc.scalar.activation(lc[:], ps_v[:], AF.Ln, bias=u_eps_cols[:, it:it + 1])
                tcc = ck.tile([C, 512], BF16, tag="tcc", name="tcc")
                nc.scalar.activation(tcc[:], lc[:], AF.Exp, scale=-0.5)
                uc = ck.tile([C, 512], F32, tag="uc", name="uc")
                nc.vector.scalar_tensor_tensor(
                    out=uc[:], in0=ps_s[:], scalar=al_cols[:, it:it + 1], in1=tcc[:],
                    op0=ALU.add, op1=ALU.mult)
                pc = ck.tile([C, 512], BF16, tag="pc", name="pc")
                nc.scalar.activation(pc[:], uc[:], AF.Exp, accum_out=den_part[:, jc:jc + 1])
                ptc = ck.tile([C, 512], BF16, tag="ptc", name="ptc")
                nc.vector.tensor_tensor(out=ptc[:], in0=pc[:], in1=tcc[:], op=ALU.mult)

                pst = ps_tp.tile([C, 512], BF16, tag="tp", name="tp")
                for t in range(4):
                    nc.tensor.transpose(pst[:, bass.ts(t, 128)], ptc[:, bass.ts(t, 128)], idb[:])
                pttc = ck.tile([C, 512], BF16, tag="pttc", name="pttc")
                nc.vector.tensor_copy(pttc[:], pst[:])
                for t in range(4):
                    jt = jc * 4 + t
                    nc.tensor.matmul(
                        pv[:], pttc[:, bass.ts(t, 128)], v_s_aug_r[:, jt, :],
                        start=(jc == 0 and t == 0), stop=(jc == JCH - 1 and t == 3))

            den = stmp.tile([C, 1], F32, tag="den", name="den")
            nc.vector.tensor_reduce(den[:], den_part[:, 0:JCH], axis=AX.X, op=ALU.add)
            rcol = stmp.tile([C, 1], F32, tag="rcol", name="rcol")
            nc.vector.reciprocal(rcol[:], den[:])
            ams = stmp.tile([C, 2], F32, tag="ams", name="ams")
            nc.scalar.copy(ams[:], pv[:, C:C + 2])
            t1 = stmp.tile([C, 1], F32, tag="t1", name="t1")
            nc.vector.scalar_tensor_tensor(
                out=t1[:], in0=ams[:, 0:1], scalar=m_r_cols[:, it:it + 1], in1=ams[:, 1:2],
                op0=ALU.mult, op1=ALU.add)
            brr = stmp.tile([C, 1], F32, tag="brr", name="brr")
            nc.vector.scalar_tensor_tensor(
                out=brr[:], in0=t1[:], scalar=-1.0, in1=rcol[:], op0=ALU.mult, op1=ALU.mult)
            x1 = stmp.tile([C, C], F32, tag="x1", name="x1")
            nc.vector.scalar_tensor_tensor(
                out=x1[:], in0=v_r_nat[:, isl], scalar=ams[:, 0:1], in1=pv[:, 0:C],
                op0=ALU.mult, op1=ALU.add)
            x2 = stmp.tile([C, C], F32R, tag="x2", name="x2")
            nc.scalar.activation(x2[:], x1[:], AF.Identity, bias=brr[:], scale=rcol[:])
            pso = ps_tp.tile([C, C], F32R, tag="tp", name="tp")
            nc.tensor.transpose(pso[:], x2[:], idf[:])
            nc.scalar.copy(outT_pre[:, isl], pso[:])

        # -------- phase 4: output projections --------
        for w, bias_col, rhs, out_d in (
            (w_proj, bp, outT_pre, outT_d),
            (w_r, br_c, v_rT, vr2T_d),
            (w_s, bs_c, v_sTo, vs2T_d),
        ):
            for c3 in range(NO // 512):
                sl = bass.ts(c3, 512)
                ps = ps_mm.tile([C, 512], F32, tag="mm", name="mm")
                nc.tensor.matmul(ps[:], w[:], rhs[:, sl], start=True, stop=True)
                ob = stmp.tile([C, 512], F32, tag="ob", name="ob")
                nc.scalar.activation(ob[:], ps[:], AF.Identity, bias=bias_col[:])
                nc.sync.dma_start(out_d[:, sl], ob[:])

    nc.compile()
    return nc


def _host_prep(inputs):
    """Returns list of 8 per-core input dicts."""
    f32 = np.float32
    x = np.ascontiguousarray(np.asarray(inputs["x"], f32))
    recv = np.asarray(inputs["receiver_val_res"], f32)
    send = np.asarray(inputs["sender_val_res"], f32)
    res_r = np.asarray(inputs["residual_receiver"], f32)
    res_s = np.asarray(inputs["residual_sender"], f32)
    mask = np.asarray(inputs["attn_mask"])
    ra = np.asarray(inputs["relation_attn"], f32)
    q_w = np.asarray(inputs["q_w"], f32)
    proj_w = np.asarray(inputs["proj_w"], f32)
    proj_b = np.asarray(inputs["proj_b"], f32)
    r_w = np.asarray(inputs["r_w"], f32)
    r_b = np.asarray(inputs["r_b"], f32)
    s_w = np.asarray(inputs["s_w"], f32)
    s_b = np.asarray(inputs["s_b"], f32)
    n_weight = np.asarray(inputs["n_weight"], f32)
    n_bias = np.asarray(inputs["n_bias"], f32)

    mem_w, recv_w, send_w = ra[:, :C], ra[:, C:2 * C], ra[:, 2 * C:]
    w_proj_eff = proj_w * n_weight[None, :]
    b_proj_eff = proj_w @ n_bias + proj_b

    cc = np.ascontiguousarray
    weights = {
        "w_send": cc(send_w.T), "w_mem": cc(mem_w.T), "w_recv": cc(recv_w.T),
        "w_qs": cc(q_w.T * SCALE), "w_proj": cc(w_proj_eff.T),
        "w_r": cc(r_w.T), "w_s": cc(s_w.T),
        "bp": cc(b_proj_eff[:, None]), "br": cc(r_b[:, None]), "bs": cc(s_b[:, None]),
        "idf": cc(np.eye(C, dtype=f32)),
        "idb": cc(np.eye(C).astype(ml_dtypes.bfloat16)),
        "oneD": np.full((C, 1), 1.0 / C, f32),
        "one": np.ones((C, 1), f32),
        "ones_row": np.ones((1, NO), f32),
    }

    in_maps = []
    for core in range(8):
        b, half = core // 2, core % 2
        i0, i1 = half * NO, (half + 1) * NO
        xb = cc(x[:, b, :].T)                      # [C, N]
        sb = cc(send[:, b, :].T)
        rsb = cc(res_s[:, b, :].T)
        m = {
            "xT": xb, "xTo": cc(xb[:, i0:i1]),
            "sendT": sb, "sendTo": cc(sb[:, i0:i1]),
            "res_sT": rsb, "res_sTo": cc(rsb[:, i0:i1]),
            "recvTo": cc(recv[i0:i1, b, :].T),
            "res_rTo": cc(res_r[i0:i1, b, :].T),
            "mask": cc(mask[b, 0, i0:i1, :].astype(np.uint8)),
        }
        m.update(weights)
        in_maps.append(m)
    return in_maps


def kernel(**inputs):
    if "nc" not in _CACHE:
        _CACHE["nc"] = _build_program()
    nc = _CACHE["nc"]
    in_maps = _host_prep(inputs)
    res = run_bass_kernel_spmd(nc, in_maps, core_ids=list(range(8)))
    out = np.zeros((N, B, C), np.float32)
    vr2 = np.zeros((N, B, C), np.float32)
    vs2 = np.zeros((N, B, C), np.float32)
    for core in range(8):
        b, half = core // 2, core % 2
        i0, i1 = half * NO, (half + 1) * NO
        r = res.results[core]
        out[i0:i1, b, :] = r["outT"].T
        vr2[i0:i1, b, :] = r["vr2T"].T
        vs2[i0:i1, b, :] = r["vs2T"].T
    return out, vr2, vs2
